# revision 14
# baseline (speedup 1.0000x reference)
"""Trainium2 Bass kernel for AngularTerms: out[p, a*8+s] = 2*f1[p,s]*f2[p,a]*fcj[p].

Self-contained: hardcodes shapes for vectors12 (2, 2000000, 3) f32 -> (2000000, 64) f32.
Data-parallel over the pair axis P across 8 NeuronCores; no collectives.

Math (per pair p, with v0, v1 the two displacement vectors; d_i = |v_i|):
  x     = 0.95*dot(v0,v1)/(d0*d1) = cos(theta)
  theta = pi/2 - arctan(x/y),  y = sqrt(1-x^2)   (y>0 since theta in [0,pi])
  f1[s] = ((1+cos(theta-Z_s))/2)^32 = exp(64*ln(sin(-a/2 + (3pi/4 - Z_s/2))))
  fcj   = (cos(pi*d0/7)*cos(pi*d1/7))^2 = qq
  2*f2[a]*fcj = exp(ln2 + ln(qq) - 2*(s01-2*ShfA_a)^2),  s01 = d0+d1
  out[p, a*8+s] = f1[s] * {2*f2[a]*fcj}

sqrt(2) is folded into d (free via the Sqrt activation's scale): then
s01' = sqrt2*s01 and u' = s01' - 2*sqrt2*ShfA gives u'^2 = 2u^2 directly,
so the whole u-path runs in fp16 tensor_tensor at DVE 2x_1p.

Schedule: three activation-table phases per group ({sqrt} {sin,arctan}
{ln,exp}), ACT stream hard-ordered by barrier markers, and groups emitted
SKEWED -- P1(0) P2(0) P1(1) P3(0) P2(1) P3(1) -- so the DVE-heavy P1 of the
next group executes under the ACT-heavy P3 (exp expansion) of the previous
one. Table loads stay at 3 per group. The u-path runs inside P3 (DVE) so
only s01 (fp16, double-buffered) crosses the P1->P3 skew boundary.

Hardware-measured notes: DVE 2x_1p needs all operands 2-byte innermost
stride +-1 (broadcast middle dims fine); innermost runs of 2 are ~3x slower
than runs of 8; GPSIMD shares the DVE SBUF port and poisons DVE throughput
(kept idle); fp16 scalar_tensor_tensor has no fast uop (plain TT used).
"""
import sys

sys.path.insert(0, "/opt/trn_rl_repo")

import numpy as np
import ml_dtypes  # noqa: F401  (bf16 numpy dtype)
from contextlib import ExitStack

import concourse.bass as bass
import concourse.tile as tile
from concourse import bacc, mybir
from concourse.bass_utils import run_bass_kernel_spmd

F32 = mybir.dt.float32
F16 = mybir.dt.float16
BF16 = mybir.dt.bfloat16
AL = mybir.AluOpType
AF = mybir.ActivationFunctionType

P_TOTAL = 2_000_000
NCORES = 8
P_CORE = P_TOTAL // NCORES      # 250,000
N = 196                          # pairs per partition per tile
T = 10                           # tiles per core
P_PAD = 128 * N * T              # 250,880
SQ2 = float(np.sqrt(2.0))

NGROUPS = 5                      # phase-groups per core (table-load batches)
NE = 7                           # ShfA columns via full-width ACT exp-expand
F2REP_BUFS = 2
USE_BARRIERS = True              # hard ACT phase barriers (kill table thrash)
SKEW = True                      # pipeline groups: P1(g+1) before P3(g)

SHFA = np.array([0.9, 1.225, 1.55, 1.875, 2.2, 2.525, 2.85, 3.175], np.float32)
SHFZ = np.array([0.19634954, 0.58904862, 0.9817477, 1.37444679,
                 1.76714587, 2.15984495, 2.55254403, 2.94524311], np.float32)

_CACHE: dict = {}


def _build_nc(N=N, T=T, ngroups=NGROUPS, ne=NE, use_barriers=USE_BARRIERS,
              f2rep_bufs=F2REP_BUFS, out_bufs=3, skew=SKEW):
    P_PAD = 128 * N * T
    TILE_PAIRS = 128 * N
    assert T % ngroups == 0
    TG = T // ngroups
    nd = 8 - ne
    nc = bacc.Bacc()
    vec = nc.declare_dram_parameter("vectors12", [2, P_PAD, 3], F32, isOutput=False)
    cst = nc.declare_dram_parameter("cst", [128, 8], F32, isOutput=False)
    cst16 = nc.declare_dram_parameter("cst16", [128, 8 * N], F16, isOutput=False)
    out = nc.declare_dram_parameter("out", [P_PAD, 64], BF16, isOutput=True)

    from concourse.bass import _add_dep_helper
    phase_acts: list = []
    prev_marker = [None]

    def act(*args, **kw):
        ins = nc.scalar.activation(*args, **kw)
        if prev_marker[0] is not None:
            _add_dep_helper(ins.ins, prev_marker[0].ins, sync=False,
                            reason="act phase fan-out")
        phase_acts.append(ins)
        return ins

    with tile.TileContext(nc) as tc, ExitStack() as ctx:
        const = ctx.enter_context(tc.tile_pool(name="const", bufs=1))
        carp = ctx.enter_context(tc.tile_pool(name="car", bufs=1))
        car2 = ctx.enter_context(tc.tile_pool(name="car2", bufs=2))
        pA = ctx.enter_context(tc.tile_pool(name="pA", bufs=2))
        sm = ctx.enter_context(tc.tile_pool(name="sm", bufs=2))
        pC = ctx.enter_context(tc.tile_pool(name="pC", bufs=2))
        pR = ctx.enter_context(tc.tile_pool(name="pR", bufs=f2rep_bufs))
        outp = ctx.enter_context(tc.tile_pool(name="outp", bufs=out_bufs))

        cstT = const.tile([128, 8], F32)
        nc.sync.dma_start(cstT[:], cst[:])
        ZC = cstT[:, 0:8]            # 3pi/4 - ShfZ/2
        A2E = const.tile([128, 8 * N], F16)   # 2*sqrt2*ShfA_a replicated over n
        nc.sync.dma_start(A2E[:], cst16[:])

        def const_scalar(val, name):
            t = const.tile([128, 1], F32, tag=name)
            nc.vector.memset(t[:], float(val))
            return t[:]

        b_pi2 = const_scalar(np.pi / 2, "pi2")
        b_one = const_scalar(1.0, "one")
        b_ln2 = const_scalar(float(np.log(2.0)), "ln2")
        dummy = const.tile([128, 1], F32, tag="dummy")
        nc.vector.memset(dummy[:], 0.0)

        def phase_barrier():
            if not use_barriers:
                return
            marker = nc.scalar.activation(dummy[:], dummy[:], AF.Copy)
            for a in phase_acts:
                _add_dep_helper(marker.ins, a.ins, sync=False,
                                reason="act phase fan-in")
            phase_acts.clear()
            prev_marker[0] = marker

        # per-group carried tiles (bufs=1 tags are shared across groups; the
        # skew keeps their write-after-read ordering legal; s01 crosses the
        # P1(g+1)-before-P3(g) boundary so it is double-buffered in car2)
        cars = {}
        for g in range(ngroups):
            car_d = carp.tile([128, 2 * N * TG], F32, tag="car_d")
            car_txy = carp.tile([128, N * TG], F32, tag="car_txy")
            car_cw = carp.tile([128, 8 * N * TG], F32, tag="car_cw")
            car_qq = carp.tile([128, N * TG], F16, tag="car_qq")
            car_s01 = car2.tile([128, N * TG], F16, tag="car_s01")
            cars[g] = dict(d=car_d, txy=car_txy, cw=car_cw, qq=car_qq,
                           s01=car_s01)

        def sl(tile_ap, w, tl):
            return tile_ap[:, tl * w: (tl + 1) * w]

        def phase1(g):
            car = cars[g]
            for tl in range(TG):
                base = (g * TG + tl) * TILE_PAIRS
                VV = pA.tile([128, 6 * N], F32, tag="VV")
                nc.sync.dma_start(
                    VV[:, : 3 * N],
                    vec[0, base: base + TILE_PAIRS, :].rearrange("(p n) c -> p (n c)", p=128),
                )
                nc.sync.dma_start(
                    VV[:, 3 * N:],
                    vec[1, base: base + TILE_PAIRS, :].rearrange("(p n) c -> p (n c)", p=128),
                )
                PR = pA.tile([128, 3 * N], F32, tag="PR")
                nc.vector.tensor_tensor(PR[:], VV[:, :3 * N], VV[:, 3 * N:], AL.mult)
                nc.vector.tensor_tensor(VV[:], VV[:], VV[:], AL.mult)  # squares in place
                RD = pA.tile([128, 3 * N], F32, tag="RD")
                nc.vector.tensor_reduce(
                    RD[:, :N], PR[:].rearrange("p (n c) -> p n c", c=3),
                    mybir.AxisListType.X, AL.add)
                nc.vector.tensor_reduce(
                    RD[:, N:], VV[:].rearrange("p (i n c) -> p (i n) c", i=2, c=3),
                    mybir.AxisListType.X, AL.add)

                d_sl = sl(car['d'], 2 * N, tl)
                act(d_sl, RD[:, N:], AF.Sqrt, scale=2.0)   # sqrt2 * d

                my = sm.tile([128, 2 * N], F32, tag="my")   # [2*d0*d1 | y]
                nc.vector.tensor_tensor(my[:, :N], d_sl[:, :N], d_sl[:, N:], AL.mult)
                x = sm.tile([128, N], F32, tag="x")
                cc = sm.tile([128, N], F32, tag="cc")
                rmy = sm.tile([128, 2 * N], F32, tag="rmy")
                nc.vector.reciprocal_approx_fast(rmy[:, :N], my[:, :N])
                nc.vector.scalar_tensor_tensor(x[:], RD[:, :N], 1.9, rmy[:, :N],
                                               AL.mult, AL.mult)
                nc.vector.scalar_tensor_tensor(cc[:], x[:], -1.0, x[:],
                                               AL.mult, AL.mult)
                act(my[:, N:], cc[:], AF.Sqrt, bias=b_one)   # y = sqrt(1-x^2)
                nc.vector.reciprocal_approx_fast(rmy[:, N:], my[:, N:])
                nc.vector.tensor_tensor(sl(car['txy'], N, tl), x[:], rmy[:, N:], AL.mult)
                # s01' = sqrt2*(d0+d1), fp16 (sole P1 product the skewed P3 uses)
                nc.vector.tensor_tensor(sl(car['s01'], N, tl),
                                        d_sl[:, :N], d_sl[:, N:], AL.add)
            phase_barrier()

        def phase2(g):
            car = cars[g]
            for tl in range(TG):
                d_sl = sl(car['d'], 2 * N, tl)
                cw_sl = sl(car['cw'], 8 * N, tl)
                S12 = sm.tile([128, 2 * N], F32, tag="S12")
                act(S12[:], d_sl, AF.Sin, bias=b_pi2, scale=float(-np.pi / 7 / SQ2))
                q = sm.tile([128, N], F32, tag="q")
                nc.vector.tensor_tensor(q[:], S12[:, :N], S12[:, N:], AL.mult)
                nc.vector.tensor_tensor(sl(car['qq'], N, tl), q[:], q[:], AL.mult)

                a8 = sm.tile([128, N], F32, tag="a8")
                act(a8[:], sl(car['txy'], N, tl), AF.Arctan)
                # G8 = -a/2 + ZC_s  -> into cw slot, then Sin in place
                cwv = cw_sl.rearrange("p (n s) -> p n s", s=8)
                ZCb = ZC[:, None, :].to_broadcast([128, N, 8])
                a8b = a8[:][:, :, None].to_broadcast([128, N, 8])
                nc.vector.scalar_tensor_tensor(cwv, a8b, -0.5, ZCb, AL.mult, AL.add)
                act(cw_sl, cw_sl, AF.Sin)
            phase_barrier()

        def phase3(g):
            car = cars[g]
            for tl in range(TG):
                base = (g * TG + tl) * TILE_PAIRS
                cw_sl = sl(car['cw'], 8 * N, tl)

                act(cw_sl, cw_sl, AF.Ln)             # lnC in place
                lnqq16 = sm.tile([128, N], F16, tag="lnqq16")
                act(lnqq16[:], sl(car['qq'], N, tl), AF.Ln)
                F1 = pC.tile([128, 8 * N], BF16, tag="F1")
                act(F1[:], cw_sl, AF.Exp, scale=64.0)

                # u-path (fp16 TT 2x_1p): u' = s01' - 2sqrt2*A; W2 = u'^2 - lnqq
                s01_sl = sl(car['s01'], N, tl)
                u16 = sm.tile([128, 8 * N], F16, tag="u16")
                s01b = s01_sl[:, None, :].to_broadcast([128, 8, N])
                A2v = A2E[:].rearrange("p (a n) -> p a n", a=8)
                u16v = u16[:].rearrange("p (a n) -> p a n", a=8)
                nc.vector.tensor_tensor(u16v, s01b, A2v, AL.subtract)
                W2 = sm.tile([128, 8 * N], F16, tag="W2")
                W2v = W2[:].rearrange("p (a n) -> p a n", a=8)
                nc.vector.tensor_tensor(W2[:], u16[:], u16[:], AL.mult)
                lnqb = lnqq16[:][:, None, :].to_broadcast([128, 8, N])
                nc.vector.tensor_tensor(W2v, W2v, lnqb, AL.subtract)

                # F2Q = exp(-W2 + ln2): full-width-8 for ne cols, narrow rest
                W2t = W2[:].rearrange("p (a n) -> p n a", a=8)
                F2R = pR.tile([128, ne * 8 * N], BF16, tag="F2R")
                F2Rv = F2R[:].rearrange("p (n a s) -> p n a s", a=ne, s=8)
                W2be = W2t[:, :, :ne, None].to_broadcast([128, N, ne, 8])
                act(F2Rv, W2be, AF.Exp, bias=b_ln2, scale=-1.0)

                OUT = outp.tile([128, 64 * N], BF16, tag="OUT")
                OUTv = OUT[:].rearrange("p (n a s) -> p n a s", a=8, s=8)
                F1v = F1[:].rearrange("p (n s) -> p n s", s=8)

                F1be = F1v[:, :, None, :].to_broadcast([128, N, ne, 8])
                nc.vector.tensor_tensor(OUTv[:, :, :ne, :], F1be, F2Rv, AL.mult)

                if nd:
                    F2n = sm.tile([128, nd * N], BF16, tag="F2n")
                    F2nv = F2n[:].rearrange("p (a n) -> p a n", a=nd)
                    act(F2nv, W2v[:, ne:, :], AF.Exp, bias=b_ln2, scale=-1.0)
                    F1bd = F1v[:, :, None, :].to_broadcast([128, N, nd, 8])
                    F2nb = F2nv.rearrange("p a n -> p n a")[:, :, :, None] \
                        .to_broadcast([128, N, nd, 8])
                    nc.vector.tensor_tensor(OUTv[:, :, ne:, :], F1bd, F2nb, AL.mult)

                nc.sync.dma_start(
                    out[base: base + TILE_PAIRS, :].rearrange("(p n) f -> p (n f)", p=128),
                    OUT[:],
                )
            phase_barrier()

        if skew:
            # P1(0) P2(0) P1(1) P3(0) P2(1) P1(2) P3(1) ... P3(NG-1)
            phase1(0)
            phase2(0)
            for g in range(1, ngroups):
                phase1(g)
                phase3(g - 1)
                phase2(g)
            phase3(ngroups - 1)
        else:
            for g in range(ngroups):
                phase1(g)
                phase2(g)
                phase3(g)

    # Restrict activation-table membership so each phase's functions resolve
    # to one set (avoids the greedy first-set binding thrashing table loads).
    import concourse.bacc as bacc_mod
    from concourse.hw_specs import get_activation_tables as _real_gat
    keep = {"sqrt_and_others", "trig_and_small", "natural_log_exp_and_others"}

    def _gat(arch):
        return {k: (v if k in keep else set()) for k, v in _real_gat(arch).items()}

    bacc_mod.get_activation_tables = _gat
    try:
        nc.compile()
    finally:
        bacc_mod.get_activation_tables = _real_gat
    return nc


def _cst_arrays(N=N):
    zc = (0.75 * np.pi - 0.5 * SHFZ).astype(np.float32)
    cst = np.broadcast_to(zc, (128, 8)).copy()
    a2 = np.repeat((2.0 * SQ2 * SHFA).astype(np.float16), N)   # [a, n] layout
    cst16 = np.broadcast_to(a2, (128, 8 * N)).copy()
    return cst, cst16


def _run(vectors12: np.ndarray, trace: bool = False):
    if "nc" not in _CACHE:
        _CACHE["nc"] = _build_nc()
    nc = _CACHE["nc"]

    v = np.ascontiguousarray(np.asarray(vectors12, dtype=np.float32))
    pad = np.zeros((2, P_PAD - P_CORE, 3), np.float32)
    pad[:, :, 0] = 1.0  # unit vectors: all downstream math well-defined
    cst, cst16 = _cst_arrays()

    in_maps = []
    for i in range(NCORES):
        shard = v[:, i * P_CORE: (i + 1) * P_CORE, :]
        shard = np.concatenate([shard, pad], axis=1)
        in_maps.append({"vectors12": np.ascontiguousarray(shard),
                        "cst": cst, "cst16": cst16})

    res = run_bass_kernel_spmd(nc, in_maps, core_ids=list(range(NCORES)),
                               trace=trace)
    out = np.empty((P_TOTAL, 64), np.float32)
    for i in range(NCORES):
        shard_out = np.asarray(res.results[i]["out"])[:P_CORE]
        out[i * P_CORE: (i + 1) * P_CORE] = shard_out.astype(np.float32)
    return out, res


def kernel(vectors12, EtaA=None, Zeta=None, ShfA=None, ShfZ=None):
    out, _ = _run(vectors12, trace=False)
    return out


# revision 15
# speedup vs baseline: 1.1419x; 1.1419x over previous
"""Trainium2 Bass kernel for AngularTerms: out[p, a*8+s] = 2*f1[p,s]*f2[p,a]*fcj[p].

Self-contained: hardcodes shapes for vectors12 (2, 2000000, 3) f32 -> (2000000, 64) f32.
Data-parallel over the pair axis P across 8 NeuronCores; no collectives.

Math (per pair p, with v0, v1 the two displacement vectors; d_i = |v_i|):
  x     = 0.95*dot(v0,v1)/(d0*d1) = cos(theta)
  theta = pi/2 - arctan(x/y),  y = sqrt(1-x^2)   (y>0 since theta in [0,pi])
  f1[s] = ((1+cos(theta-Z_s))/2)^32 = exp(64*ln(sin(-a/2 + (3pi/4 - Z_s/2))))
  fcj   = (cos(pi*d0/7)*cos(pi*d1/7))^2 = qq
  2*f2[a]*fcj = exp(ln2 + ln(qq) - 2*(s01-2*ShfA_a)^2),  s01 = d0+d1
  out[p, a*8+s] = f1[s] * {2*f2[a]*fcj}

sqrt(2) is folded into d (free via the Sqrt activation's scale): then
s01' = sqrt2*s01 and u' = s01' - 2*sqrt2*ShfA gives u'^2 = 2u^2 directly,
so the whole u-path runs in fp16 tensor_tensor at DVE 2x_1p.

Schedule: three activation-table phases per group ({sqrt} {sin,arctan}
{ln,exp}), ACT stream hard-ordered by barrier markers, and groups emitted
SKEWED -- P1(0) P2(0) P1(1) P3(0) P2(1) P3(1) -- so the DVE-heavy P1 of the
next group executes under the ACT-heavy P3 (exp expansion) of the previous
one. Table loads stay at 3 per group. The u-path runs inside P3 (DVE) so
only s01 (fp16, double-buffered) crosses the P1->P3 skew boundary.

Hardware-measured notes: DVE 2x_1p needs all operands 2-byte innermost
stride +-1 (broadcast middle dims fine); innermost runs of 2 are ~3x slower
than runs of 8; GPSIMD shares the DVE SBUF port and poisons DVE throughput
(kept idle); fp16 scalar_tensor_tensor has no fast uop (plain TT used).
"""
import sys

sys.path.insert(0, "/opt/trn_rl_repo")

import numpy as np
import ml_dtypes  # noqa: F401  (bf16 numpy dtype)
from contextlib import ExitStack

import concourse.bass as bass
import concourse.tile as tile
from concourse import bacc, mybir
from concourse.bass_utils import run_bass_kernel_spmd

F32 = mybir.dt.float32
F16 = mybir.dt.float16
BF16 = mybir.dt.bfloat16
AL = mybir.AluOpType
AF = mybir.ActivationFunctionType

P_TOTAL = 2_000_000
NCORES = 8
P_CORE = P_TOTAL // NCORES      # 250,000
N = 196                          # pairs per partition per tile
T = 10                           # tiles per core
P_PAD = 128 * N * T              # 250,880
SQ2 = float(np.sqrt(2.0))

NGROUPS = 2                      # phase-groups per core (table-load batches)
NE = 7                           # ShfA columns via full-width ACT exp-expand
F2REP_BUFS = 2
USE_BARRIERS = True              # hard ACT phase barriers (kill table thrash)
SKEW = True                      # pipeline groups: P1(g+1) before P3(g)

SHFA = np.array([0.9, 1.225, 1.55, 1.875, 2.2, 2.525, 2.85, 3.175], np.float32)
SHFZ = np.array([0.19634954, 0.58904862, 0.9817477, 1.37444679,
                 1.76714587, 2.15984495, 2.55254403, 2.94524311], np.float32)

_CACHE: dict = {}


def _build_nc(N=N, T=T, ngroups=NGROUPS, ne=NE, use_barriers=USE_BARRIERS,
              f2rep_bufs=F2REP_BUFS, out_bufs=2, skew=SKEW, nhalves=2):
    P_PAD = 128 * N * T
    TILE_PAIRS = 128 * N
    assert T % ngroups == 0
    TG = T // ngroups
    nd = 8 - ne
    nc = bacc.Bacc()
    vec = nc.declare_dram_parameter("vectors12", [2, P_PAD, 3], F32, isOutput=False)
    cst = nc.declare_dram_parameter("cst", [128, 8], F32, isOutput=False)
    cst16 = nc.declare_dram_parameter("cst16", [128, 8 * N], F16, isOutput=False)
    out = nc.declare_dram_parameter("out", [P_PAD, 64], BF16, isOutput=True)

    from concourse.bass import _add_dep_helper
    phase_acts: list = []
    prev_marker = [None]

    def act(*args, **kw):
        ins = nc.scalar.activation(*args, **kw)
        if prev_marker[0] is not None:
            _add_dep_helper(ins.ins, prev_marker[0].ins, sync=False,
                            reason="act phase fan-out")
        phase_acts.append(ins)
        return ins

    with tile.TileContext(nc) as tc, ExitStack() as ctx:
        const = ctx.enter_context(tc.tile_pool(name="const", bufs=1))
        carp = ctx.enter_context(tc.tile_pool(name="car", bufs=1))
        car2 = ctx.enter_context(tc.tile_pool(name="car2", bufs=2))
        pA = ctx.enter_context(tc.tile_pool(name="pA", bufs=2))
        sm = ctx.enter_context(tc.tile_pool(name="sm", bufs=2))
        pC = ctx.enter_context(tc.tile_pool(name="pC", bufs=2))
        pR = ctx.enter_context(tc.tile_pool(name="pR", bufs=f2rep_bufs))
        outp = ctx.enter_context(tc.tile_pool(name="outp", bufs=out_bufs))

        cstT = const.tile([128, 8], F32)
        nc.sync.dma_start(cstT[:], cst[:])
        ZC = cstT[:, 0:8]            # 3pi/4 - ShfZ/2
        A2E = const.tile([128, 8 * N], F16)   # 2*sqrt2*ShfA_a replicated over n
        nc.sync.dma_start(A2E[:], cst16[:])

        def const_scalar(val, name):
            t = const.tile([128, 1], F32, tag=name)
            nc.vector.memset(t[:], float(val))
            return t[:]

        b_pi2 = const_scalar(np.pi / 2, "pi2")
        b_one = const_scalar(1.0, "one")
        b_ln2 = const_scalar(float(np.log(2.0)), "ln2")
        dummy = const.tile([128, 1], F32, tag="dummy")
        nc.vector.memset(dummy[:], 0.0)

        def phase_barrier():
            if not use_barriers:
                return
            marker = nc.scalar.activation(dummy[:], dummy[:], AF.Copy)
            for a in phase_acts:
                _add_dep_helper(marker.ins, a.ins, sync=False,
                                reason="act phase fan-in")
            phase_acts.clear()
            prev_marker[0] = marker

        # per-group carried tiles (bufs=1 tags are shared across groups; the
        # skew keeps their write-after-read ordering legal; s01 crosses the
        # P1(g+1)-before-P3(g) boundary so it is double-buffered in car2)
        cars = {}
        for g in range(ngroups):
            car_d = carp.tile([128, 2 * N * TG], F32, tag="car_d")
            car_txy = carp.tile([128, N * TG], F32, tag="car_txy")
            car_cw = carp.tile([128, 8 * N * TG], F32, tag="car_cw")
            car_qq = carp.tile([128, N * TG], F16, tag="car_qq")
            car_s01 = car2.tile([128, N * TG], F16, tag="car_s01")
            cars[g] = dict(d=car_d, txy=car_txy, cw=car_cw, qq=car_qq,
                           s01=car_s01)

        def sl(tile_ap, w, tl):
            return tile_ap[:, tl * w: (tl + 1) * w]

        def phase1(g):
            car = cars[g]
            for tl in range(TG):
                base = (g * TG + tl) * TILE_PAIRS
                VV = pA.tile([128, 6 * N], F32, tag="VV")
                nc.sync.dma_start(
                    VV[:, : 3 * N],
                    vec[0, base: base + TILE_PAIRS, :].rearrange("(p n) c -> p (n c)", p=128),
                )
                nc.sync.dma_start(
                    VV[:, 3 * N:],
                    vec[1, base: base + TILE_PAIRS, :].rearrange("(p n) c -> p (n c)", p=128),
                )
                PR = pA.tile([128, 3 * N], F32, tag="PR")
                nc.vector.tensor_tensor(PR[:], VV[:, :3 * N], VV[:, 3 * N:], AL.mult)
                nc.vector.tensor_tensor(VV[:], VV[:], VV[:], AL.mult)  # squares in place
                RD = pA.tile([128, 3 * N], F32, tag="RD")
                nc.vector.tensor_reduce(
                    RD[:, :N], PR[:].rearrange("p (n c) -> p n c", c=3),
                    mybir.AxisListType.X, AL.add)
                nc.vector.tensor_reduce(
                    RD[:, N:], VV[:].rearrange("p (i n c) -> p (i n) c", i=2, c=3),
                    mybir.AxisListType.X, AL.add)

                d_sl = sl(car['d'], 2 * N, tl)
                act(d_sl, RD[:, N:], AF.Sqrt, scale=2.0)   # sqrt2 * d

                my = sm.tile([128, 2 * N], F32, tag="my")   # [2*d0*d1 | y]
                nc.vector.tensor_tensor(my[:, :N], d_sl[:, :N], d_sl[:, N:], AL.mult)
                x = sm.tile([128, N], F32, tag="x")
                cc = sm.tile([128, N], F32, tag="cc")
                rmy = sm.tile([128, 2 * N], F32, tag="rmy")
                nc.vector.reciprocal_approx_fast(rmy[:, :N], my[:, :N])
                nc.vector.scalar_tensor_tensor(x[:], RD[:, :N], 1.9, rmy[:, :N],
                                               AL.mult, AL.mult)
                nc.vector.scalar_tensor_tensor(cc[:], x[:], -1.0, x[:],
                                               AL.mult, AL.mult)
                act(my[:, N:], cc[:], AF.Sqrt, bias=b_one)   # y = sqrt(1-x^2)
                nc.vector.reciprocal_approx_fast(rmy[:, N:], my[:, N:])
                nc.vector.tensor_tensor(sl(car['txy'], N, tl), x[:], rmy[:, N:], AL.mult)
                # s01' = sqrt2*(d0+d1), fp16 (sole P1 product the skewed P3 uses)
                nc.vector.tensor_tensor(sl(car['s01'], N, tl),
                                        d_sl[:, :N], d_sl[:, N:], AL.add)
            phase_barrier()

        def phase2(g):
            car = cars[g]
            for tl in range(TG):
                d_sl = sl(car['d'], 2 * N, tl)
                cw_sl = sl(car['cw'], 8 * N, tl)
                S12 = sm.tile([128, 2 * N], F32, tag="S12")
                act(S12[:], d_sl, AF.Sin, bias=b_pi2, scale=float(-np.pi / 7 / SQ2))
                q = sm.tile([128, N], F32, tag="q")
                nc.vector.tensor_tensor(q[:], S12[:, :N], S12[:, N:], AL.mult)
                nc.vector.tensor_tensor(sl(car['qq'], N, tl), q[:], q[:], AL.mult)

                a8 = sm.tile([128, N], F32, tag="a8")
                act(a8[:], sl(car['txy'], N, tl), AF.Arctan)
                # G8 = -a/2 + ZC_s  -> into cw slot, then Sin in place
                cwv = cw_sl.rearrange("p (n s) -> p n s", s=8)
                ZCb = ZC[:, None, :].to_broadcast([128, N, 8])
                a8b = a8[:][:, :, None].to_broadcast([128, N, 8])
                nc.vector.scalar_tensor_tensor(cwv, a8b, -0.5, ZCb, AL.mult, AL.add)
                act(cw_sl, cw_sl, AF.Sin)
            phase_barrier()

        def phase3(g):
            car = cars[g]
            for tl in range(TG):
                base = (g * TG + tl) * TILE_PAIRS
                cw_sl = sl(car['cw'], 8 * N, tl)

                act(cw_sl, cw_sl, AF.Ln)             # lnC in place
                lnqq16 = sm.tile([128, N], F16, tag="lnqq16")
                act(lnqq16[:], sl(car['qq'], N, tl), AF.Ln)
                F1 = pC.tile([128, 8 * N], BF16, tag="F1")
                act(F1[:], cw_sl, AF.Exp, scale=64.0)

                # u-path (fp16 TT 2x_1p): u' = s01' - 2sqrt2*A; W2 = u'^2 - lnqq
                s01_sl = sl(car['s01'], N, tl)
                u16 = sm.tile([128, 8 * N], F16, tag="u16")
                s01b = s01_sl[:, None, :].to_broadcast([128, 8, N])
                A2v = A2E[:].rearrange("p (a n) -> p a n", a=8)
                u16v = u16[:].rearrange("p (a n) -> p a n", a=8)
                nc.vector.tensor_tensor(u16v, s01b, A2v, AL.subtract)
                W2 = sm.tile([128, 8 * N], F16, tag="W2")
                W2v = W2[:].rearrange("p (a n) -> p a n", a=8)
                nc.vector.tensor_tensor(W2[:], u16[:], u16[:], AL.mult)
                lnqb = lnqq16[:][:, None, :].to_broadcast([128, 8, N])
                nc.vector.tensor_tensor(W2v, W2v, lnqb, AL.subtract)

                # F2Q = exp(-W2 + ln2): full-width-8 for ne cols, narrow
                # rest; emitted in halves along n so DVE's OUT multiply on
                # half h overlaps ACT's exp on half h+1
                W2t = W2[:].rearrange("p (a n) -> p n a", a=8)
                F2R = pR.tile([128, ne * 8 * N], BF16, tag="F2R")
                F2Rv = F2R[:].rearrange("p (n a s) -> p n a s", a=ne, s=8)
                OUT = outp.tile([128, 64 * N], BF16, tag="OUT")
                OUTv = OUT[:].rearrange("p (n a s) -> p n a s", a=8, s=8)
                F1v = F1[:].rearrange("p (n s) -> p n s", s=8)
                if nd:
                    F2n = sm.tile([128, nd * N], BF16, tag="F2n")
                    F2nv = F2n[:].rearrange("p (a n) -> p a n", a=nd)
                NH = N // nhalves
                for h in range(nhalves):
                    ns = slice(h * NH, (h + 1) * NH)
                    W2be = W2t[:, ns, :ne, None].to_broadcast([128, NH, ne, 8])
                    act(F2Rv[:, ns], W2be, AF.Exp, bias=b_ln2, scale=-1.0)
                    if nd:
                        act(F2nv[:, :, ns], W2v[:, ne:, ns], AF.Exp,
                            bias=b_ln2, scale=-1.0)
                    F1be = F1v[:, ns, None, :].to_broadcast([128, NH, ne, 8])
                    nc.vector.tensor_tensor(OUTv[:, ns, :ne, :], F1be,
                                            F2Rv[:, ns], AL.mult)
                    if nd:
                        F1bd = F1v[:, ns, None, :].to_broadcast([128, NH, nd, 8])
                        F2nb = F2nv.rearrange("p a n -> p n a")[:, ns, :, None] \
                            .to_broadcast([128, NH, nd, 8])
                        nc.vector.tensor_tensor(OUTv[:, ns, ne:, :], F1bd,
                                                F2nb, AL.mult)

                nc.sync.dma_start(
                    out[base: base + TILE_PAIRS, :].rearrange("(p n) f -> p (n f)", p=128),
                    OUT[:],
                )
            phase_barrier()

        if skew:
            # P1(0) P2(0) P1(1) P3(0) P2(1) P1(2) P3(1) ... P3(NG-1)
            phase1(0)
            phase2(0)
            for g in range(1, ngroups):
                phase1(g)
                phase3(g - 1)
                phase2(g)
            phase3(ngroups - 1)
        else:
            for g in range(ngroups):
                phase1(g)
                phase2(g)
                phase3(g)

    # Restrict activation-table membership so each phase's functions resolve
    # to one set (avoids the greedy first-set binding thrashing table loads).
    import concourse.bacc as bacc_mod
    from concourse.hw_specs import get_activation_tables as _real_gat
    keep = {"sqrt_and_others", "trig_and_small", "natural_log_exp_and_others"}

    def _gat(arch):
        return {k: (v if k in keep else set()) for k, v in _real_gat(arch).items()}

    bacc_mod.get_activation_tables = _gat
    try:
        nc.compile()
    finally:
        bacc_mod.get_activation_tables = _real_gat
    return nc


def _cst_arrays(N=N):
    zc = (0.75 * np.pi - 0.5 * SHFZ).astype(np.float32)
    cst = np.broadcast_to(zc, (128, 8)).copy()
    a2 = np.repeat((2.0 * SQ2 * SHFA).astype(np.float16), N)   # [a, n] layout
    cst16 = np.broadcast_to(a2, (128, 8 * N)).copy()
    return cst, cst16


def _run(vectors12: np.ndarray, trace: bool = False):
    if "nc" not in _CACHE:
        _CACHE["nc"] = _build_nc()
    nc = _CACHE["nc"]

    v = np.ascontiguousarray(np.asarray(vectors12, dtype=np.float32))
    pad = np.zeros((2, P_PAD - P_CORE, 3), np.float32)
    pad[:, :, 0] = 1.0  # unit vectors: all downstream math well-defined
    cst, cst16 = _cst_arrays()

    in_maps = []
    for i in range(NCORES):
        shard = v[:, i * P_CORE: (i + 1) * P_CORE, :]
        shard = np.concatenate([shard, pad], axis=1)
        in_maps.append({"vectors12": np.ascontiguousarray(shard),
                        "cst": cst, "cst16": cst16})

    res = run_bass_kernel_spmd(nc, in_maps, core_ids=list(range(NCORES)),
                               trace=trace)
    out = np.empty((P_TOTAL, 64), np.float32)
    for i in range(NCORES):
        shard_out = np.asarray(res.results[i]["out"])[:P_CORE]
        out[i * P_CORE: (i + 1) * P_CORE] = shard_out.astype(np.float32)
    return out, res


def kernel(vectors12, EtaA=None, Zeta=None, ShfA=None, ShfZ=None):
    out, _ = _run(vectors12, trace=False)
    return out


# revision 17
# speedup vs baseline: 1.1545x; 1.0111x over previous
"""Trainium2 Bass kernel for AngularTerms: out[p, a*8+s] = 2*f1[p,s]*f2[p,a]*fcj[p].

Self-contained: hardcodes shapes for vectors12 (2, 2000000, 3) f32 -> (2000000, 64) f32.
Data-parallel over the pair axis P across 8 NeuronCores; no collectives.

Math (per pair p, with v0, v1 the two displacement vectors; d_i = |v_i|):
  x     = 0.95*dot(v0,v1)/(d0*d1) = cos(theta)
  theta = pi/2 - arctan(x/y),  y = sqrt(1-x^2)   (y>0 since theta in [0,pi])
  f1[s] = ((1+cos(theta-Z_s))/2)^32 = exp(64*ln(sin(-a/2 + (3pi/4 - Z_s/2))))
  fcj   = (cos(pi*d0/7)*cos(pi*d1/7))^2 = qq
  2*f2[a]*fcj = exp(ln2 + ln(qq) - 2*(s01-2*ShfA_a)^2),  s01 = d0+d1
  out[p, a*8+s] = f1[s] * {2*f2[a]*fcj}

sqrt(2) is folded into d (free via the Sqrt activation's scale): then
s01' = sqrt2*s01 and u' = s01' - 2*sqrt2*ShfA gives u'^2 = 2u^2 directly,
so the whole u-path runs in fp16 tensor_tensor at DVE 2x_1p.

Schedule: three activation-table phases per group ({sqrt} {sin,arctan}
{ln,exp}), ACT stream hard-ordered by barrier markers, and groups emitted
SKEWED -- P1(0) P2(0) P1(1) P3(0) P2(1) P3(1) -- so the DVE-heavy P1 of the
next group executes under the ACT-heavy P3 (exp expansion) of the previous
one. Table loads stay at 3 per group. The u-path runs inside P3 (DVE) so
only s01 (fp16, double-buffered) crosses the P1->P3 skew boundary.

Hardware-measured notes: DVE 2x_1p needs all operands 2-byte innermost
stride +-1 (broadcast middle dims fine); innermost runs of 2 are ~3x slower
than runs of 8; GPSIMD shares the DVE SBUF port and poisons DVE throughput
(kept idle); fp16 scalar_tensor_tensor has no fast uop (plain TT used).
"""
import sys

sys.path.insert(0, "/opt/trn_rl_repo")

import numpy as np
import ml_dtypes  # noqa: F401  (bf16 numpy dtype)
from contextlib import ExitStack

import concourse.bass as bass
import concourse.tile as tile
from concourse import bacc, mybir
from concourse.bass_utils import run_bass_kernel_spmd

F32 = mybir.dt.float32
F16 = mybir.dt.float16
BF16 = mybir.dt.bfloat16
AL = mybir.AluOpType
AF = mybir.ActivationFunctionType

P_TOTAL = 2_000_000
NCORES = 8
P_CORE = P_TOTAL // NCORES      # 250,000
N = 196                          # pairs per partition per tile
T = 10                           # tiles per core
P_PAD = 128 * N * T              # 250,880
SQ2 = float(np.sqrt(2.0))

NGROUPS = 1                      # phase-groups per core (table-load batches)
NE = 7                           # ShfA columns via full-width ACT exp-expand
F2REP_BUFS = 2
USE_BARRIERS = False             # scheduler freedom beats table thrash
SKEW = True                      # pipeline groups: P1(g+1) before P3(g)

SHFA = np.array([0.9, 1.225, 1.55, 1.875, 2.2, 2.525, 2.85, 3.175], np.float32)
SHFZ = np.array([0.19634954, 0.58904862, 0.9817477, 1.37444679,
                 1.76714587, 2.15984495, 2.55254403, 2.94524311], np.float32)

_CACHE: dict = {}


def _build_nc(N=N, T=T, ngroups=NGROUPS, ne=NE, use_barriers=USE_BARRIERS,
              f2rep_bufs=F2REP_BUFS, out_bufs=2, skew=SKEW, nhalves=2):
    P_PAD = 128 * N * T
    TILE_PAIRS = 128 * N
    assert T % ngroups == 0
    TG = T // ngroups
    nd = 8 - ne
    nc = bacc.Bacc()
    vec = nc.declare_dram_parameter("vectors12", [2, P_PAD, 3], F32, isOutput=False)
    cst = nc.declare_dram_parameter("cst", [128, 8], F32, isOutput=False)
    cst16 = nc.declare_dram_parameter("cst16", [128, 8 * N], F16, isOutput=False)
    out = nc.declare_dram_parameter("out", [P_PAD, 64], BF16, isOutput=True)

    from concourse.bass import _add_dep_helper
    phase_acts: list = []
    prev_marker = [None]

    def act(*args, **kw):
        ins = nc.scalar.activation(*args, **kw)
        if prev_marker[0] is not None:
            _add_dep_helper(ins.ins, prev_marker[0].ins, sync=False,
                            reason="act phase fan-out")
        phase_acts.append(ins)
        return ins

    with tile.TileContext(nc) as tc, ExitStack() as ctx:
        const = ctx.enter_context(tc.tile_pool(name="const", bufs=1))
        carp = ctx.enter_context(tc.tile_pool(name="car", bufs=1))
        car2 = ctx.enter_context(tc.tile_pool(name="car2", bufs=2))
        pA = ctx.enter_context(tc.tile_pool(name="pA", bufs=2))
        sm = ctx.enter_context(tc.tile_pool(name="sm", bufs=2))
        pC = ctx.enter_context(tc.tile_pool(name="pC", bufs=2))
        pR = ctx.enter_context(tc.tile_pool(name="pR", bufs=f2rep_bufs))
        outp = ctx.enter_context(tc.tile_pool(name="outp", bufs=out_bufs))

        cstT = const.tile([128, 8], F32)
        nc.sync.dma_start(cstT[:], cst[:])
        ZC = cstT[:, 0:8]            # 3pi/4 - ShfZ/2
        A2E = const.tile([128, 8 * N], F16)   # 2*sqrt2*ShfA_a replicated over n
        nc.sync.dma_start(A2E[:], cst16[:])

        def const_scalar(val, name):
            t = const.tile([128, 1], F32, tag=name)
            nc.vector.memset(t[:], float(val))
            return t[:]

        b_pi2 = const_scalar(np.pi / 2, "pi2")
        b_one = const_scalar(1.0, "one")
        b_ln2 = const_scalar(float(np.log(2.0)), "ln2")
        dummy = const.tile([128, 1], F32, tag="dummy")
        nc.vector.memset(dummy[:], 0.0)

        def phase_barrier():
            if not use_barriers:
                return
            marker = nc.scalar.activation(dummy[:], dummy[:], AF.Copy)
            for a in phase_acts:
                _add_dep_helper(marker.ins, a.ins, sync=False,
                                reason="act phase fan-in")
            phase_acts.clear()
            prev_marker[0] = marker

        # per-group carried tiles (bufs=1 tags are shared across groups; the
        # skew keeps their write-after-read ordering legal; s01 crosses the
        # P1(g+1)-before-P3(g) boundary so it is double-buffered in car2)
        cars = {}
        for g in range(ngroups):
            car_d = carp.tile([128, 2 * N * TG], F32, tag="car_d")
            car_txy = carp.tile([128, N * TG], F32, tag="car_txy")
            car_cw = carp.tile([128, 8 * N * TG], F32, tag="car_cw")
            car_qq = carp.tile([128, N * TG], F16, tag="car_qq")
            car_s01 = (car2 if ngroups > 1 else carp).tile([128, N * TG], F16, tag="car_s01")
            cars[g] = dict(d=car_d, txy=car_txy, cw=car_cw, qq=car_qq,
                           s01=car_s01)

        def sl(tile_ap, w, tl):
            return tile_ap[:, tl * w: (tl + 1) * w]

        def phase1(g):
            car = cars[g]
            for tl in range(TG):
                base = (g * TG + tl) * TILE_PAIRS
                VV = pA.tile([128, 6 * N], F32, tag="VV")
                nc.sync.dma_start(
                    VV[:, : 3 * N],
                    vec[0, base: base + TILE_PAIRS, :].rearrange("(p n) c -> p (n c)", p=128),
                )
                nc.sync.dma_start(
                    VV[:, 3 * N:],
                    vec[1, base: base + TILE_PAIRS, :].rearrange("(p n) c -> p (n c)", p=128),
                )
                PR = pA.tile([128, 3 * N], F32, tag="PR")
                nc.vector.tensor_tensor(PR[:], VV[:, :3 * N], VV[:, 3 * N:], AL.mult)
                nc.vector.tensor_tensor(VV[:], VV[:], VV[:], AL.mult)  # squares in place
                RD = pA.tile([128, 3 * N], F32, tag="RD")
                nc.vector.tensor_reduce(
                    RD[:, :N], PR[:].rearrange("p (n c) -> p n c", c=3),
                    mybir.AxisListType.X, AL.add)
                nc.vector.tensor_reduce(
                    RD[:, N:], VV[:].rearrange("p (i n c) -> p (i n) c", i=2, c=3),
                    mybir.AxisListType.X, AL.add)

                d_sl = sl(car['d'], 2 * N, tl)
                act(d_sl, RD[:, N:], AF.Sqrt, scale=2.0)   # sqrt2 * d

                my = sm.tile([128, 2 * N], F32, tag="my")   # [2*d0*d1 | y]
                nc.vector.tensor_tensor(my[:, :N], d_sl[:, :N], d_sl[:, N:], AL.mult)
                x = sm.tile([128, N], F32, tag="x")
                cc = sm.tile([128, N], F32, tag="cc")
                rmy = sm.tile([128, 2 * N], F32, tag="rmy")
                nc.vector.reciprocal_approx_fast(rmy[:, :N], my[:, :N])
                nc.vector.scalar_tensor_tensor(x[:], RD[:, :N], 1.9, rmy[:, :N],
                                               AL.mult, AL.mult)
                nc.vector.scalar_tensor_tensor(cc[:], x[:], -1.0, x[:],
                                               AL.mult, AL.mult)
                act(my[:, N:], cc[:], AF.Sqrt, bias=b_one)   # y = sqrt(1-x^2)
                nc.vector.reciprocal_approx_fast(rmy[:, N:], my[:, N:])
                nc.vector.tensor_tensor(sl(car['txy'], N, tl), x[:], rmy[:, N:], AL.mult)
                # s01' = sqrt2*(d0+d1), fp16 (sole P1 product the skewed P3 uses)
                nc.vector.tensor_tensor(sl(car['s01'], N, tl),
                                        d_sl[:, :N], d_sl[:, N:], AL.add)
            phase_barrier()

        def phase2(g):
            car = cars[g]
            for tl in range(TG):
                d_sl = sl(car['d'], 2 * N, tl)
                cw_sl = sl(car['cw'], 8 * N, tl)
                S12 = sm.tile([128, 2 * N], F32, tag="S12")
                act(S12[:], d_sl, AF.Sin, bias=b_pi2, scale=float(-np.pi / 7 / SQ2))
                q = sm.tile([128, N], F32, tag="q")
                nc.vector.tensor_tensor(q[:], S12[:, :N], S12[:, N:], AL.mult)
                nc.vector.tensor_tensor(sl(car['qq'], N, tl), q[:], q[:], AL.mult)

                a8 = sm.tile([128, N], F32, tag="a8")
                act(a8[:], sl(car['txy'], N, tl), AF.Arctan)
                # G8 = -a/2 + ZC_s  -> into cw slot, then Sin in place
                cwv = cw_sl.rearrange("p (n s) -> p n s", s=8)
                ZCb = ZC[:, None, :].to_broadcast([128, N, 8])
                a8b = a8[:][:, :, None].to_broadcast([128, N, 8])
                nc.vector.scalar_tensor_tensor(cwv, a8b, -0.5, ZCb, AL.mult, AL.add)
                act(cw_sl, cw_sl, AF.Sin)
            phase_barrier()

        def phase3(g):
            car = cars[g]
            for tl in range(TG):
                base = (g * TG + tl) * TILE_PAIRS
                cw_sl = sl(car['cw'], 8 * N, tl)

                act(cw_sl, cw_sl, AF.Ln)             # lnC in place
                lnqq16 = sm.tile([128, N], F16, tag="lnqq16")
                act(lnqq16[:], sl(car['qq'], N, tl), AF.Ln)
                F1 = pC.tile([128, 8 * N], BF16, tag="F1")
                act(F1[:], cw_sl, AF.Exp, scale=64.0)

                # u-path (fp16 TT 2x_1p): u' = s01' - 2sqrt2*A; W2 = u'^2 - lnqq
                s01_sl = sl(car['s01'], N, tl)
                u16 = sm.tile([128, 8 * N], F16, tag="u16")
                s01b = s01_sl[:, None, :].to_broadcast([128, 8, N])
                A2v = A2E[:].rearrange("p (a n) -> p a n", a=8)
                u16v = u16[:].rearrange("p (a n) -> p a n", a=8)
                nc.vector.tensor_tensor(u16v, s01b, A2v, AL.subtract)
                W2 = sm.tile([128, 8 * N], F16, tag="W2")
                W2v = W2[:].rearrange("p (a n) -> p a n", a=8)
                nc.vector.tensor_tensor(W2[:], u16[:], u16[:], AL.mult)
                lnqb = lnqq16[:][:, None, :].to_broadcast([128, 8, N])
                nc.vector.tensor_tensor(W2v, W2v, lnqb, AL.subtract)

                # F2Q = exp(-W2 + ln2): full-width-8 for ne cols, narrow
                # rest; emitted in halves along n (half-sized tiles) so DVE's
                # OUT multiply on half h overlaps ACT's exp on half h+1, and
                # the output DMA streams per half
                W2t = W2[:].rearrange("p (a n) -> p n a", a=8)
                F1v = F1[:].rearrange("p (n s) -> p n s", s=8)
                if nd:
                    F2n = sm.tile([128, nd * N], BF16, tag="F2n")
                    F2nv = F2n[:].rearrange("p (a n) -> p a n", a=nd)
                NH = N // nhalves
                for h in range(nhalves):
                    ns = slice(h * NH, (h + 1) * NH)
                    F2R = pR.tile([128, ne * 8 * NH], BF16, tag="F2R")
                    F2Rv = F2R[:].rearrange("p (n a s) -> p n a s", a=ne, s=8)
                    OUT = outp.tile([128, 64 * NH], BF16, tag="OUT")
                    OUTv = OUT[:].rearrange("p (n a s) -> p n a s", a=8, s=8)
                    W2be = W2t[:, ns, :ne, None].to_broadcast([128, NH, ne, 8])
                    act(F2Rv, W2be, AF.Exp, bias=b_ln2, scale=-1.0)
                    if nd:
                        act(F2nv[:, :, ns], W2v[:, ne:, ns], AF.Exp,
                            bias=b_ln2, scale=-1.0)
                    F1be = F1v[:, ns, None, :].to_broadcast([128, NH, ne, 8])
                    nc.vector.tensor_tensor(OUTv[:, :, :ne, :], F1be,
                                            F2Rv, AL.mult)
                    if nd:
                        F1bd = F1v[:, ns, None, :].to_broadcast([128, NH, nd, 8])
                        F2nb = F2nv.rearrange("p a n -> p n a")[:, ns, :, None] \
                            .to_broadcast([128, NH, nd, 8])
                        nc.vector.tensor_tensor(OUTv[:, :, ne:, :], F1bd,
                                                F2nb, AL.mult)
                    full = out[base: base + TILE_PAIRS, :].rearrange(
                        "(p n) f -> p (n f)", p=128)
                    nc.sync.dma_start(
                        full[:, h * NH * 64: (h * NH + NH) * 64],
                        OUT[:],
                    )
            phase_barrier()

        if skew:
            # P1(0) P2(0) P1(1) P3(0) P2(1) P1(2) P3(1) ... P3(NG-1)
            phase1(0)
            phase2(0)
            for g in range(1, ngroups):
                phase1(g)
                phase3(g - 1)
                phase2(g)
            phase3(ngroups - 1)
        else:
            for g in range(ngroups):
                phase1(g)
                phase2(g)
                phase3(g)

    # Restrict activation-table membership so each phase's functions resolve
    # to one set (avoids the greedy first-set binding thrashing table loads).
    import concourse.bacc as bacc_mod
    from concourse.hw_specs import get_activation_tables as _real_gat
    keep = {"sqrt_and_others", "trig_and_small", "natural_log_exp_and_others"}

    def _gat(arch):
        return {k: (v if k in keep else set()) for k, v in _real_gat(arch).items()}

    bacc_mod.get_activation_tables = _gat
    try:
        nc.compile()
    finally:
        bacc_mod.get_activation_tables = _real_gat
    return nc


def _cst_arrays(N=N):
    zc = (0.75 * np.pi - 0.5 * SHFZ).astype(np.float32)
    cst = np.broadcast_to(zc, (128, 8)).copy()
    a2 = np.repeat((2.0 * SQ2 * SHFA).astype(np.float16), N)   # [a, n] layout
    cst16 = np.broadcast_to(a2, (128, 8 * N)).copy()
    return cst, cst16


def _run(vectors12: np.ndarray, trace: bool = False):
    if "nc" not in _CACHE:
        _CACHE["nc"] = _build_nc()
    nc = _CACHE["nc"]

    v = np.ascontiguousarray(np.asarray(vectors12, dtype=np.float32))
    pad = np.zeros((2, P_PAD - P_CORE, 3), np.float32)
    pad[:, :, 0] = 1.0  # unit vectors: all downstream math well-defined
    cst, cst16 = _cst_arrays()

    in_maps = []
    for i in range(NCORES):
        shard = v[:, i * P_CORE: (i + 1) * P_CORE, :]
        shard = np.concatenate([shard, pad], axis=1)
        in_maps.append({"vectors12": np.ascontiguousarray(shard),
                        "cst": cst, "cst16": cst16})

    res = run_bass_kernel_spmd(nc, in_maps, core_ids=list(range(NCORES)),
                               trace=trace)
    out = np.empty((P_TOTAL, 64), np.float32)
    for i in range(NCORES):
        shard_out = np.asarray(res.results[i]["out"])[:P_CORE]
        out[i * P_CORE: (i + 1) * P_CORE] = shard_out.astype(np.float32)
    return out, res


def kernel(vectors12, EtaA=None, Zeta=None, ShfA=None, ShfZ=None):
    out, _ = _run(vectors12, trace=False)
    return out
